# revision 19
# baseline (speedup 1.0000x reference)
"""HSTU block kernel for Trainium2, 8-core data-parallel over batch.

Key structure:
  - All matmuls use bf16/fp16 operands (1 PE cycle/row vs 4 for fp32).
  - x is shipped as xT [D, N] bf16 (stats + proj moving operand) and
    row-major fp32 with b_o pre-added (residual).
  - proj is produced transposed (projT [E, N] fp16) for u/q/k; v row-major
    fp16 (stationary operand of attn@v), with its bias applied via an extra
    1-partition matmul row and the 1/N * pad scale on the DVE eviction.
  - rel-bias = pos[n-m] + ts_w[bucket(log dt)]: reconstructed with
    threshold-compare/accumulate passes (piecewise-constant in log dt).
    The near field (j = n-m < 256) is computed in a SKEWED layout
    (host ships a rolling window tsqs[p, g*256+j] = tsq[128g+p+j]) where
    each j-column spans few buckets, so the per-chunk threshold ranges are
    narrow.  Un-skewed via a DRAM round-trip (strided re-read).  Cells that
    are causally masked or out of range are filled with -60000 so that
    silu(qk + bias) == 0 there - no separate masking pass needed.
    The far field (n >= m0+256) is done in [m, n] layout per row-tile with
    DP-optimized column segmentation.
  - qk PSUM chunks are [128, <=1024] (2 banks) so one Silu eviction covers
    two matmul column-halves; the bias is preloaded into PSUM with an
    identity matmul (fp16, 1 cyc/row).
  - attn@v exploits causality at 128-column granularity.
"""

import sys

sys.path.insert(0, "/opt/trn_rl_repo")

import numpy as np

import concourse.bass as bass
import concourse.tile as tile
import concourse.mybir as mybir
from concourse import bacc
from concourse.ap import AP
from concourse.masks import make_identity

B, N, D = 8, 1024, 512
H, DV, DQ = 8, 64, 64
E = 2 * H * DV + 2 * H * DQ  # 2048
EPS = 1e-5
P = 128
NT = N // P  # 8 row tiles
J = 256  # near-field skew window
F32 = mybir.dt.float32
F16 = mybir.dt.float16
BF16 = mybir.dt.bfloat16
NPBF16 = mybir.dt.np(mybir.dt.bfloat16)
NPF16 = np.float16
TH = 2.0 * 0.301  # thresholds on y = ln(d^2)
FILL = -60000.0

_cache = {}


def _bucket(d):
    d = np.maximum(np.abs(d), 1).astype(np.float64)
    return np.clip((np.log(d) / 0.301).astype(np.int64), 0, 128)


def _dp_segments(atoms, atom_w, pass_cost_fn):
    """atoms: list of (kmin, kmax) per atom (or None). Returns segments
    (a0, a1, kmin, kmax) minimizing sum of (kmax-kmin)*pass_cost(width)."""
    n = len(atoms)
    INF = float("inf")
    dp = [0.0] + [INF] * n
    prev = [0] * (n + 1)
    for i in range(1, n + 1):
        for j0 in range(max(0, i - 4), i):
            ks = [a for a in atoms[j0:i] if a is not None]
            if not ks:
                c = 0.0
            else:
                kmin = min(a[0] for a in ks)
                kmax = max(a[1] for a in ks)
                c = (kmax - kmin) * pass_cost_fn((i - j0) * atom_w)
            if dp[j0] + c < dp[i]:
                dp[i] = dp[j0] + c
                prev[i] = j0
    segs = []
    i = n
    while i > 0:
        j0 = prev[i]
        ks = [a for a in atoms[j0:i] if a is not None]
        if ks:
            kmin = min(a[0] for a in ks)
            kmax = max(a[1] for a in ks)
        else:
            kmin = kmax = 0
        segs.append((j0 * atom_w, i * atom_w, int(kmin), int(kmax)))
        i = j0
    return segs[::-1]


def _plan(ts, tsq):
    """Data-dependent threshold plans, unioned across batches."""
    # ---- skew (near field) plan ----
    m = np.arange(N)
    idx = np.minimum(m[:, None] + np.arange(J)[None, :], N - 1)  # [N, J]
    dsk = tsq[:, idx] - ts[:, :, None]  # [B, N, J]
    bsk = _bucket(dsk)
    pp = m % P
    gg = m // P
    Lg = np.where(gg <= 6, 255, 127)
    valid = (pp[:, None] + np.arange(J)[None, :]) <= Lg[:, None]  # [N, J]
    atom_w = 16
    atoms = []
    for a in range(J // atom_w):
        sl = bsk[:, :, a * atom_w:(a + 1) * atom_w]
        v = np.broadcast_to(valid[None, :, a * atom_w:(a + 1) * atom_w], sl.shape)
        vals = sl[v]
        atoms.append((int(vals.min()), int(vals.max())) if vals.size else None)

    def skew_cost(w):
        return (8 * w * 1.0417 * 0.25 + 110) + (8 * w * 1.0417 * 0.5 + 110)

    skew_segs = _dp_segments(atoms, atom_w, skew_cost)

    # ---- far field plan (per row-tile) ----
    def far_cost(w):
        return (w * 1.0417 * 0.25 + 110) + (w * 1.0417 * 0.5 + 110)

    far_plans = {}
    for r in range(NT):
        n0f = P * r + 2 * P
        if n0f >= N:
            far_plans[r] = []
            continue
        na = (N - n0f) // P
        atoms_f = []
        for a in range(na):
            lo, hi = n0f + P * a, n0f + P * (a + 1)
            dmin = int((tsq[:, lo] - ts[:, P * r + P - 1]).min())
            dmax = int((tsq[:, hi - 1] - ts[:, P * r]).max())
            atoms_f.append((int(_bucket(np.array([max(dmin, 0)]))[0]),
                            int(_bucket(np.array([dmax]))[0])))
        segs = _dp_segments(atoms_f, P, far_cost)
        far_plans[r] = [(n0f + a0, n0f + a1, kmin, kmax)
                        for (a0, a1, kmin, kmax) in segs]
    return skew_segs, far_plans


def _build(ts_w_np, skew_segs, far_plans, needs_gamma_a, needs_padout):
    nc = bacc.Bacc()
    far_w = [max(0, N - (P * r + 2 * P)) for r in range(NT)]
    far_off = np.concatenate([[0], np.cumsum(far_w)]).astype(int)
    FW = int(far_off[-1])  # 2688

    d = {}
    for name, shape, dt in [
        ("xT_p", [P, 4 * N], BF16),
        ("xr_p", [P, NT * D], F32),
        ("uvqk_p", [P, 4 * E], BF16),
        ("wo_p", [P, 4 * D], BF16),
        ("bUv_row", [1, D], BF16),
        ("tsqs", [P, NT * J], F32),
        ("ntsk", [P, NT], F32),
        ("tsq_rep", [P, N], F32),
        ("near_init", [P, NT * J], F16),
        ("far_init", [P, FW], F16),
        ("smalls", [P, 48], F32),
    ]:
        d[name] = nc.dram_tensor(name, shape, dt, kind="ExternalInput")
    skew_sc = nc.dram_tensor("skew_sc", [P * NT * J], F16, kind="Internal")
    out_t = nc.dram_tensor("out", [N, D], F32, kind="ExternalOutput")

    tsw = ts_w_np.astype(np.float64)
    cks = [float(tsw[k] - tsw[k - 1]) for k in range(1, 129)]

    from contextlib import ExitStack
    with tile.TileContext(nc) as tc, ExitStack() as ctx:
        io = ctx.enter_context(tc.tile_pool(name="io", bufs=1))
        pools = ctx.enter_context(tc.tile_pool(name="work", bufs=2))
        rpool = ctx.enter_context(tc.tile_pool(name="resid", bufs=8))
        kpool = ctx.enter_context(tc.tile_pool(name="kpool", bufs=2))
        psqk = ctx.enter_context(tc.tile_pool(name="psqk", bufs=3, space="PSUM"))
        pstat = ctx.enter_context(tc.tile_pool(name="pstat", bufs=1, space="PSUM"))

        AT = mybir.AluOpType
        AF = mybir.ActivationFunctionType

        # ---- input DMAs, ordered by first use ----
        xT = io.tile([P, 4 * N], BF16, tag="xT")
        nc.sync.dma_start(xT[:, 0:2 * N], d["xT_p"][:, 0:2 * N])
        nc.sync.dma_start(xT[:, 2 * N:4 * N], d["xT_p"][:, 2 * N:4 * N])
        tsqs = io.tile([P, NT * J], F32, tag="tsqs")
        nc.sync.dma_start(tsqs[:], d["tsqs"][:])
        ntsk = io.tile([P, NT], F32, tag="ntsk")
        nc.sync.dma_start(ntsk[:], d["ntsk"][:])
        tsq_rep = io.tile([P, N], F32, tag="tsqr")
        nc.sync.dma_start(tsq_rep[:], d["tsq_rep"][:])
        near = io.tile([P, NT * J], F16, tag="near")
        nc.sync.dma_start(near[:], d["near_init"][:])
        facc = io.tile([P, FW], F16, tag="facc")
        nc.sync.dma_start(facc[:], d["far_init"][:])
        smalls = io.tile([P, 48], F32, tag="smalls")
        nc.sync.dma_start(smalls[:], d["smalls"][:])
        bUv = io.tile([1, D], BF16, tag="bUv")
        nc.sync.dma_start(bUv[:], d["bUv_row"][:])
        uvqk = io.tile([P, 4 * E], BF16, tag="uvqk")
        nc.sync.dma_start(uvqk[:], d["uvqk_p"][:])
        wo = io.tile([P, 4 * D], BF16, tag="wo")
        # smalls layout: bU_col [0:16], vscale [16:24], padout [24:32],
        # ga [32:36], bb [36:40]

        ident = io.tile([P, P], F16, tag="ident")
        make_identity(nc, ident[:])
        ots = []
        for t in range(NT):
            ot = rpool.tile([P, D], F32, tag="wot", name=f"ot{t}")
            ots.append(ot)
        ones_col = io.tile([P, 1], BF16, tag="ones_col")
        nc.vector.memset(ones_col[:], 1.0)
        ones_row16 = io.tile([1, P], F16, tag="ones_row16")
        nc.vector.memset(ones_row16[:], 1.0)
        ones_rowb = io.tile([1, P], BF16, tag="ones_rowb")
        nc.vector.memset(ones_rowb[:], 1.0)

        # ---- rel-bias prep on Act: d2 = (tsqs - ts)^2, y = ln(d2) ----
        d2 = io.tile([P, FW], F32, tag="d2")
        for g in range(NT):
            nc.scalar.activation(d2[:, J * g:J * g + J], tsqs[:, J * g:J * g + J],
                                 AF.Square, bias=ntsk[:, g:g + 1], scale=1.0)
        yk = io.tile([P, NT * J], F16, tag="yk")
        nc.scalar.activation(yk[:], d2[:, 0:NT * J], AF.Ln)
        for r in range(NT):
            if far_w[r] == 0:
                continue
            n0f = P * r + 2 * P
            nc.scalar.activation(d2[:, far_off[r]:far_off[r + 1]],
                                 tsq_rep[:, n0f:N], AF.Square,
                                 bias=ntsk[:, r:r + 1], scale=1.0)
        yf = io.tile([P, FW], F16, tag="yf")
        nc.scalar.activation(yf[:], d2[:], AF.Ln)

        # ---- layernorm stats of x ----
        stats = pstat.tile([P, 2 * D], F32, tag="stats", name="stats")
        for s in range(4):
            sq = pools.tile([P, N], BF16, tag="wsq", name="sq")
            nc.vector.tensor_tensor(sq[:], xT[:, N * s:N * s + N],
                                    xT[:, N * s:N * s + N], AT.mult)
            for c in range(2):
                nc.tensor.matmul(stats[0:1, 512 * c:512 * c + 512], ones_col[:],
                                 xT[:, N * s + 512 * c:N * s + 512 * c + 512],
                                 start=(s == 0), stop=(s == 3))
                nc.tensor.matmul(stats[32:33, 512 * c:512 * c + 512], ones_col[:],
                                 sq[:, 512 * c:512 * c + 512],
                                 start=(s == 0), stop=(s == 3))
        stg = io.tile([1, 2 * N], F16, tag="stg")
        tmp1 = pools.tile([1, N], F32, tag="wrow", name="tmp1")
        mu2 = pools.tile([1, N], F32, tag="wrow", name="mu2")
        for c in range(2):
            nc.vector.tensor_scalar_mul(stg[:, 512 * c:512 * c + 512],
                                        stats[0:1, 512 * c:512 * c + 512], 1.0 / D)
            nc.vector.tensor_scalar(tmp1[:, 512 * c:512 * c + 512],
                                    stats[32:33, 512 * c:512 * c + 512],
                                    1.0 / D, EPS, AT.mult, AT.add)
        nc.vector.tensor_tensor(mu2[:], stg[:, 0:N], stg[:, 0:N], AT.mult)
        nc.vector.tensor_tensor(tmp1[:], tmp1[:], mu2[:], AT.subtract)
        nc.scalar.activation(tmp1[:], tmp1[:], AF.Sqrt)
        with nc.allow_low_precision(reason="fp16 rstd is fine"):
            nc.vector.reciprocal(stg[:, N:2 * N], tmp1[:])
        # replicate mu, rs to [P, 2N] fp16 via PE broadcast
        murs = io.tile([P, 2 * N], F16, tag="murs")
        for c in range(4):
            pt = psqk.tile([P, 2 * 512], F32, tag="qk", name="rep")
            nc.tensor.matmul(pt[:, 0:512], ones_row16[:],
                             stg[:, 512 * c:512 * c + 512],
                             start=True, stop=True)
            nc.scalar.copy(out=murs[:, 512 * c:512 * c + 512], in_=pt[:, 0:512])
        # xnT = (xT - mu) * rs  (in place, bf16)
        for s in range(4):
            nc.vector.tensor_tensor(xT[:, N * s:N * s + N], xT[:, N * s:N * s + N],
                                    murs[:, 0:N], AT.subtract)
            nc.vector.tensor_tensor(xT[:, N * s:N * s + N], xT[:, N * s:N * s + N],
                                    murs[:, N:2 * N], AT.mult)

        # ---- skew threshold passes (DVE) ----
        def near_ap(tile_, j0, w, ng):
            a = tile_[:]
            return AP(a.tensor, a.offset + j0, [[NT * J, P], [J, ng], [1, w]])

        for (j0, j1, kmin, kmax) in skew_segs:
            w = j1 - j0
            ng = 7 if j0 >= 128 else 8
            for k in range(kmin + 1, kmax + 1):
                t = kpool.tile([P, 512], F16, tag="kt")
                ta = t[:]
                tap = AP(ta.tensor, ta.offset, [[512, P], [w, ng], [1, w]])
                nc.vector.tensor_scalar(tap, near_ap(yk, j0, w, ng),
                                        float(TH * k), cks[k - 1],
                                        AT.is_ge, AT.mult)
                nc.vector.tensor_tensor(near_ap(near, j0, w, ng),
                                        near_ap(near, j0, w, ng),
                                        tap, AT.add)
        # far threshold passes (r<2 on DVE; r>=2 offloaded to idle GPSIMD,
        # with a separate scratch ring so the two chains run concurrently)
        for r in range(NT):
            eng = nc.vector if r < 2 else nc.gpsimd
            tg = "kt" if r < 2 else "ktp"
            for (a0, a1, kmin, kmax) in far_plans[r]:
                o0 = int(far_off[r] + (a0 - (P * r + 2 * P)))
                w = a1 - a0
                for k in range(kmin + 1, kmax + 1):
                    t = kpool.tile([P, 512], F16, tag=tg, name="tthr")
                    eng.tensor_scalar(t[:, :w], yf[:, o0:o0 + w],
                                      float(TH * k), cks[k - 1],
                                      AT.is_ge, AT.mult)
                    eng.tensor_tensor(facc[:, o0:o0 + w], facc[:, o0:o0 + w],
                                      t[:, :w], AT.add)

        # corner fill: groups 0..6 keep p+j<=255; group 7 keep p+j<=127
        a06 = near[:]
        nc.gpsimd.affine_select(
            out=AP(a06.tensor, a06.offset, [[NT * J, P], [J, 7], [1, J]]),
            in_=AP(a06.tensor, a06.offset, [[NT * J, P], [J, 7], [1, J]]),
            pattern=[[0, 7], [-1, J]], compare_op=AT.is_ge, fill=FILL,
            base=255, channel_multiplier=-1)
        nc.gpsimd.affine_select(
            out=near[:, 7 * J:8 * J], in_=near[:, 7 * J:8 * J],
            pattern=[[-1, J]], compare_op=AT.is_ge, fill=FILL,
            base=127, channel_multiplier=-1)

        # prefetch residual tiles on the idle Pool DMA queue
        for t in range(NT):
            nc.gpsimd.dma_start(ots[t][:], d["xr_p"][:, D * t:D * t + D])

        # unskew via DRAM round-trip
        sc = skew_sc[:].rearrange("(a b) -> a b", a=P)
        nc.sync.dma_start(sc, near[:])
        near_mn = io.tile([P, NT * J], F16, tag="near_mn")
        scf = skew_sc[:]
        nc.sync.dma_start(
            AP(near_mn[:].tensor, near_mn[:].offset, [[NT * J, P], [J, NT], [1, J]]),
            AP(scf.tensor, scf.offset, [[NT * J - 1, P], [J, NT], [1, J]]))

        # ---- projT for q,k,u tiles (order: qk first); v row-major ----
        projT = {}
        for t in [8, 12, 9, 13, 10, 14, 11, 15, 0, 1, 2, 3]:
            projT[t] = io.tile([P, N], F16, tag=f"pT{t}", name=f"pT{t}")
            for c in range(2):
                pt = psqk.tile([P, 2 * 512], F32, tag="qk", name="ptp")
                for s in range(4):
                    nc.tensor.matmul(pt[:, 0:512],
                                     uvqk[:, E * s + P * t:E * s + P * t + P],
                                     xT[:, N * s + 512 * c:N * s + 512 * c + 512],
                                     start=(s == 0), stop=(s == 3))
                nc.scalar.activation(projT[t][:, 512 * c:512 * c + 512],
                                     pt[:, 0:512],
                                     AF.Silu, bias=smalls[:, t:t + 1], scale=1.0)
        vt = [io.tile([P, D], F16, tag=f"v{r}", name=f"v{r}") for r in range(NT)]
        for r in range(NT):
            pt = psqk.tile([P, 2 * 512], F32, tag="qk", name="ptv")
            for s in range(4):
                nc.tensor.matmul(pt[:, 0:512],
                                 xT[:, N * s + P * r:N * s + P * r + P],
                                 uvqk[:, E * s + 512:E * s + 1024],
                                 start=(s == 0), stop=False)
            nc.tensor.matmul(pt[:, 0:512], ones_rowb[:], bUv[:],
                             start=False, stop=True)
            sv = pools.tile([P, D], F16, tag="wsv", name="sv")
            nc.scalar.activation(sv[:], pt[:, 0:512], AF.Silu)
            nc.vector.tensor_scalar(vt[r][:], sv[:], smalls[:, 16 + r:17 + r],
                                    None, AT.mult)

        # ---- attention per head ----
        qksil2 = [[io.tile([P, N], F16, tag=f"qs{i}_{r}", name=f"qs{i}_{r}")
                   for r in range(NT)] for i in range(2)]
        attnT = [io.tile([P, N], BF16, tag=f"aT{t}", name=f"aT{t}") for t in range(4)]

        def attnv(h, c):
            pq = 64 * (h % 2)
            qs = qksil2[h % 2]
            pa = stats2[64:128, 512 * (h % 2):512 * (h % 2) + 512]
            first = True
            for c1 in range(4 * c, 4 * c + 4):
                col = P * (c1 - 4 * c)
                for r in range(c1 + 1):
                    last = (c1 == 4 * c + 3) and (r == c1)
                    nc.tensor.matmul(pa[:, col:col + P],
                                     vt[r][:, 64 * h:64 * h + 64],
                                     qs[r][:, P * c1:P * c1 + P],
                                     start=first, stop=last,
                                     skip_group_check=True)
                    first = False
            at = attnT[h // 2]
            if h >= 6:
                nc.scalar.copy(out=at[pq:pq + 64, 512 * c:512 * c + 512],
                               in_=pa[:, 0:512])
            else:
                nc.vector.tensor_copy(out=at[pq:pq + 64, 512 * c:512 * c + 512],
                                      in_=pa[:, 0:512])

        stats2 = pstat.tile([P, 2 * D], F32, tag="stats", name="stats2")

        def qk_tile(h, r):
            qt = projT[8 + h // 2]
            kt = projT[12 + h // 2]
            pq = 64 * (h % 2)
            m0 = P * r
            w = N - m0
            wn = min(2 * P, N - m0)  # near width (256, or 128 for r=7)
            pt = psqk.tile([P, 2 * 512], F32, tag="qk", name="pt")
            # bias preload. start=True zeroes the addressed partitions'
            # whole PSUM bank, so each 512-col bank gets start=True on its
            # FIRST piece only; later pieces accumulate onto the zeroed
            # remainder.  Pieces are also split at bank boundaries.
            started = set()

            def _preload(c0, c1_, src_ap):
                bank = c0 // 512
                st = bank not in started
                started.add(bank)
                nc.tensor.matmul(pt[:, c0:c1_], ident[:], src_ap,
                                 start=st, stop=False, skip_group_check=True)

            _preload(0, wn, near_mn[:, J * r:J * r + wn])
            for (a0, a1, kmin, kmax) in far_plans[r]:
                s0 = a0
                while s0 < a1:
                    nxt_bank = m0 + (((s0 - m0) // 512) + 1) * 512
                    s1 = min(s0 + 512, a1, nxt_bank)
                    o0 = int(far_off[r] + (s0 - (m0 + 2 * P)))
                    _preload(s0 - m0, s1 - m0, facc[:, o0:o0 + (s1 - s0)])
                    s0 = s1
            # qk matmuls in <=512 column pieces
            q0 = m0
            while q0 < N:
                q1 = min(q0 + 512, N)
                nc.tensor.matmul(pt[:, q0 - m0:q1 - m0],
                                 kt[pq:pq + 64, m0:m0 + P],
                                 qt[pq:pq + 64, q0:q1],
                                 start=False, stop=True, skip_group_check=True)
                q0 = q1
            nc.scalar.activation(qksil2[h % 2][r][:, m0:N], pt[:, 0:w], AF.Silu)

        for hp in range(H // 2):
            ha, hb = 2 * hp, 2 * hp + 1
            for r in range(NT):
                qk_tile(ha, r)
                qk_tile(hb, r)
                if r == 3:
                    attnv(ha, 0)
                    attnv(hb, 0)
            attnv(ha, 1)
            attnv(hb, 1)
            # this attnT s-tile is complete; fold its LN-a stats in now
            s = hp
            for c in range(2):
                nc.tensor.matmul(stats2[0:1, 512 * c:512 * c + 512],
                                 ones_col[:],
                                 attnT[s][:, 512 * c:512 * c + 512],
                                 start=(s == 0), stop=(s == 3),
                                 skip_group_check=True)
                sqa = pools.tile([P, 512], BF16, tag="wsqa", name="sqa")
                nc.vector.tensor_tensor(sqa[:],
                                        attnT[s][:, 512 * c:512 * c + 512],
                                        attnT[s][:, 512 * c:512 * c + 512],
                                        AT.mult)
                nc.tensor.matmul(stats2[32:33, 512 * c:512 * c + 512],
                                 ones_col[:], sqa[:],
                                 start=(s == 0), stop=(s == 3),
                                 skip_group_check=True)

        # ---- layernorm of attn (over E=512, partition dim) ----
        nc.sync.dma_start(wo[:], d["wo_p"][:])
        stg2 = io.tile([1, 2 * N], F16, tag="stg2")
        tmpa = pools.tile([1, N], F32, tag="wrow", name="tmpa")
        mua2 = pools.tile([1, N], F32, tag="wrow", name="mua2")
        for c in range(2):
            nc.vector.tensor_scalar_mul(stg2[:, 512 * c:512 * c + 512],
                                        stats2[0:1, 512 * c:512 * c + 512], 1.0 / D)
            nc.vector.tensor_scalar(tmpa[:, 512 * c:512 * c + 512],
                                    stats2[32:33, 512 * c:512 * c + 512],
                                    1.0 / D, EPS, AT.mult, AT.add)
        nc.vector.tensor_tensor(mua2[:], stg2[:, 0:N], stg2[:, 0:N], AT.mult)
        nc.vector.tensor_tensor(tmpa[:], tmpa[:], mua2[:], AT.subtract)
        nc.scalar.activation(tmpa[:], tmpa[:], AF.Sqrt)
        with nc.allow_low_precision(reason="fp16 rstd is fine"):
            nc.vector.reciprocal(stg2[:, N:2 * N], tmpa[:])
        muars = io.tile([P, 2 * N], F16, tag="muars")
        for c in range(4):
            pt = psqk.tile([P, 2 * 512], F32, tag="qk", name="rep")
            nc.tensor.matmul(pt[:, 0:512], ones_row16[:],
                             stg2[:, 512 * c:512 * c + 512],
                             start=True, stop=True)
            nc.scalar.copy(out=muars[:, 512 * c:512 * c + 512], in_=pt[:, 0:512])
        # prod = u * LN_a(attn) in attnT layout (bf16), chunked so the
        # first outproj tiles can start before the whole tail finishes
        for c in range(2):
            for s in range(4):
                sl = slice(512 * c, 512 * c + 512)
                nc.vector.tensor_tensor(attnT[s][:, sl], attnT[s][:, sl],
                                        muars[:, 512 * c:512 * c + 512],
                                        AT.subtract)
                nc.vector.tensor_tensor(attnT[s][:, sl], attnT[s][:, sl],
                                        muars[:, N + 512 * c:N + 512 * c + 512],
                                        AT.mult)
                if needs_gamma_a:
                    nc.vector.tensor_scalar(attnT[s][:, sl], attnT[s][:, sl],
                                            smalls[:, 32 + s:33 + s],
                                            smalls[:, 36 + s:37 + s],
                                            AT.mult, AT.add)
                nc.vector.tensor_tensor(attnT[s][:, sl], attnT[s][:, sl],
                                        projT[s][:, sl], AT.mult)

        # ---- output projection + residual (xr already includes b_o) ----
        for t in range(NT):
            po = psqk.tile([P, 2 * 512], F32, tag="qk", name="outp")
            for s in range(4):
                nc.tensor.matmul(po[:, 0:512], attnT[s][:, P * t:P * t + P],
                                 wo[:, D * s:D * s + D],
                                 start=(s == 0), stop=(s == 3))
            ot = ots[t]
            nc.vector.tensor_tensor(ot[:], po[:, 0:512], ot[:], AT.add)
            if needs_padout:
                nc.vector.tensor_scalar(ot[:], ot[:], smalls[:, 24 + t:25 + t],
                                        None, AT.mult)
            nc.sync.dma_start(out_t[P * t:P * t + P, :], ot[:])

    nc.compile()
    return nc


def _prep_inputs(inputs):
    x = np.asarray(inputs["x"], dtype=np.float32)
    ts = np.asarray(inputs["timestamps"]).astype(np.int64)
    pad = np.asarray(inputs["pad_mask"]).astype(np.float32)
    uvqk = np.asarray(inputs["uvqk"], dtype=np.float32)
    W_o = np.asarray(inputs["W_o"], dtype=np.float32)
    b_o = np.asarray(inputs["b_o"], dtype=np.float32)
    gx = np.asarray(inputs["gamma_x"], dtype=np.float32)
    bx = np.asarray(inputs["beta_x"], dtype=np.float32)
    ga = np.asarray(inputs["gamma_a"], dtype=np.float32)
    ba = np.asarray(inputs["beta_a"], dtype=np.float32)
    ts_w = np.asarray(inputs["ts_w"], dtype=np.float32)
    pos_w = np.asarray(inputs["pos_w"], dtype=np.float32)

    tsq = np.concatenate([ts[:, 1:], ts[:, -1:]], axis=1)  # [B, N]
    skew_segs, far_plans = _plan(ts, tsq)
    needs_gamma_a = bool(np.any(ga != 1.0) or np.any(ba != 0.0))
    needs_padout = bool(np.any(pad != 0.0))

    uvqk_g = (uvqk * gx[:, None]).astype(NPBF16)  # [D, E]
    uvqk_p = np.ascontiguousarray(
        uvqk_g.reshape(4, P, E).transpose(1, 0, 2).reshape(P, 4 * E))
    bU = bx @ uvqk  # [E]
    bU_col = np.ascontiguousarray(bU.reshape(E // P, P).T)  # [P, 16]
    wo_p = np.ascontiguousarray(
        W_o.astype(NPBF16).reshape(4, P, D).transpose(1, 0, 2).reshape(P, 4 * D))

    far_w = [max(0, N - (P * r + 2 * P)) for r in range(NT)]
    far_off = np.concatenate([[0], np.cumsum(far_w)]).astype(int)
    FW = int(far_off[-1])

    # near_init: col g*J + j -> pos_w[j + N-1] + ts_w[kmin(skew seg of j)]
    ninit_row = np.zeros(NT * J, np.float32)
    for g in range(NT):
        ninit_row[g * J:(g + 1) * J] = pos_w[(N - 1) - np.arange(J)]
    for (j0, j1, kmin, kmax) in skew_segs:
        for g in range(NT):
            ninit_row[g * J + j0:g * J + j1] += ts_w[kmin]
    near_init = np.broadcast_to(ninit_row.astype(NPF16), (P, NT * J)).copy()

    # far_init: per row-tile [m, n] layout
    far_init = np.zeros((P, FW), NPF16)
    for r in range(NT):
        if far_w[r] == 0:
            continue
        m = P * r + np.arange(P)[:, None]
        n0f = P * r + 2 * P
        nn = np.arange(n0f, N)[None, :]
        block = pos_w[m - nn + (N - 1)].astype(np.float32)
        for (a0, a1, kmin, kmax) in far_plans[r]:
            block[:, a0 - n0f:a1 - n0f] += ts_w[kmin]
        far_init[:, far_off[r]:far_off[r + 1]] = block.astype(NPF16)

    # tsqs rolling window index (per batch)
    m = np.arange(N)
    idx = np.minimum(m[:, None] + np.arange(J)[None, :], N - 1)  # [N, J]

    smalls_shared = np.zeros((P, 48), np.float32)
    smalls_shared[:, 0:16] = bU_col
    smalls_shared[:, 32:36] = ga.reshape(4, P).T
    smalls_shared[:, 36:40] = ba.reshape(4, P).T

    per_core = []
    for b in range(B):
        tsqs_b = tsq[b][idx].astype(np.float32)  # [N, J]
        tsqs_p = np.ascontiguousarray(
            tsqs_b.reshape(NT, P, J).transpose(1, 0, 2).reshape(P, NT * J))
        sm = smalls_shared.copy()
        sm[:, 16:24] = ((1.0 - pad[b]) / N).reshape(NT, P).T
        sm[:, 24:32] = (1.0 - pad[b]).reshape(NT, P).T
        per_core.append({
            "xT_p": np.ascontiguousarray(
                x[b].T.astype(NPBF16).reshape(4, P, N).transpose(1, 0, 2)
                .reshape(P, 4 * N)),
            "xr_p": np.ascontiguousarray(
                (x[b] + b_o[None, :]).reshape(NT, P, D).transpose(1, 0, 2)
                .reshape(P, NT * D)),
            "uvqk_p": uvqk_p,
            "wo_p": wo_p,
            "bUv_row": np.ascontiguousarray(bU[512:1024].reshape(1, D)).astype(NPBF16),
            "tsqs": tsqs_p,
            "ntsk": np.ascontiguousarray(-ts[b].astype(np.float32).reshape(NT, P).T),
            "tsq_rep": np.broadcast_to(tsq[b].astype(np.float32), (P, N)).copy(),
            "near_init": near_init,
            "far_init": far_init,
            "smalls": sm,
        })
    return per_core, (skew_segs, far_plans, needs_gamma_a, needs_padout, ts_w)


def kernel(**inputs):
    from concourse.bass_utils import run_bass_kernel_spmd

    per_core, (skew_segs, far_plans, nga, npo, ts_w) = _prep_inputs(inputs)
    key = (tuple(skew_segs), tuple((r, tuple(v)) for r, v in far_plans.items()),
           nga, npo, ts_w.tobytes())
    if key not in _cache:
        _cache.clear()
        _cache[key] = _build(ts_w, skew_segs, far_plans, nga, npo)
    nc = _cache[key]
    res = run_bass_kernel_spmd(nc, per_core, list(range(B)))
    out = np.stack([res.results[b]["out"] for b in range(B)], axis=0)
    return out.astype(np.float32)


# revision 20
# speedup vs baseline: 1.0034x; 1.0034x over previous
"""HSTU block kernel for Trainium2, 8-core data-parallel over batch.

Key structure:
  - All matmuls use bf16/fp16 operands (1 PE cycle/row vs 4 for fp32).
  - x is shipped as xT [D, N] bf16 (stats + proj moving operand) and
    row-major fp32 with b_o pre-added (residual).
  - proj is produced transposed (projT [E, N] fp16) for u/q/k; v row-major
    fp16 (stationary operand of attn@v), with its bias applied via an extra
    1-partition matmul row and the 1/N * pad scale on the DVE eviction.
  - rel-bias = pos[n-m] + ts_w[bucket(log dt)]: reconstructed with
    threshold-compare/accumulate passes (piecewise-constant in log dt).
    The near field (j = n-m < 256) is computed in a SKEWED layout
    (host ships a rolling window tsqs[p, g*256+j] = tsq[128g+p+j]) where
    each j-column spans few buckets, so the per-chunk threshold ranges are
    narrow.  Un-skewed via a DRAM round-trip (strided re-read).  Cells that
    are causally masked or out of range are filled with -60000 so that
    silu(qk + bias) == 0 there - no separate masking pass needed.
    The far field (n >= m0+256) is done in [m, n] layout per row-tile with
    DP-optimized column segmentation.
  - qk PSUM chunks are [128, <=1024] (2 banks) so one Silu eviction covers
    two matmul column-halves; the bias is preloaded into PSUM with an
    identity matmul (fp16, 1 cyc/row).
  - attn@v exploits causality at 128-column granularity.
"""

import sys

sys.path.insert(0, "/opt/trn_rl_repo")

import numpy as np

import concourse.bass as bass
import concourse.tile as tile
import concourse.mybir as mybir
from concourse import bacc
from concourse.ap import AP
from concourse.masks import make_identity

B, N, D = 8, 1024, 512
H, DV, DQ = 8, 64, 64
E = 2 * H * DV + 2 * H * DQ  # 2048
EPS = 1e-5
P = 128
NT = N // P  # 8 row tiles
J = 256  # near-field skew window
F32 = mybir.dt.float32
F16 = mybir.dt.float16
BF16 = mybir.dt.bfloat16
NPBF16 = mybir.dt.np(mybir.dt.bfloat16)
NPF16 = np.float16
TH = 2.0 * 0.301  # thresholds on y = ln(d^2)
FILL = -60000.0

_cache = {}


def _bucket(d):
    d = np.maximum(np.abs(d), 1).astype(np.float64)
    return np.clip((np.log(d) / 0.301).astype(np.int64), 0, 128)


def _dp_segments(atoms, atom_w, pass_cost_fn):
    """atoms: list of (kmin, kmax) per atom (or None). Returns segments
    (a0, a1, kmin, kmax) minimizing sum of (kmax-kmin)*pass_cost(width)."""
    n = len(atoms)
    INF = float("inf")
    dp = [0.0] + [INF] * n
    prev = [0] * (n + 1)
    for i in range(1, n + 1):
        for j0 in range(max(0, i - 4), i):
            ks = [a for a in atoms[j0:i] if a is not None]
            if not ks:
                c = 0.0
            else:
                kmin = min(a[0] for a in ks)
                kmax = max(a[1] for a in ks)
                c = (kmax - kmin) * pass_cost_fn((i - j0) * atom_w)
            if dp[j0] + c < dp[i]:
                dp[i] = dp[j0] + c
                prev[i] = j0
    segs = []
    i = n
    while i > 0:
        j0 = prev[i]
        ks = [a for a in atoms[j0:i] if a is not None]
        if ks:
            kmin = min(a[0] for a in ks)
            kmax = max(a[1] for a in ks)
        else:
            kmin = kmax = 0
        segs.append((j0 * atom_w, i * atom_w, int(kmin), int(kmax)))
        i = j0
    return segs[::-1]


def _plan(ts, tsq):
    """Data-dependent threshold plans, unioned across batches."""
    # ---- skew (near field) plan ----
    m = np.arange(N)
    idx = np.minimum(m[:, None] + np.arange(J)[None, :], N - 1)  # [N, J]
    dsk = tsq[:, idx] - ts[:, :, None]  # [B, N, J]
    bsk = _bucket(dsk)
    pp = m % P
    gg = m // P
    Lg = np.where(gg <= 6, 255, 127)
    valid = (pp[:, None] + np.arange(J)[None, :]) <= Lg[:, None]  # [N, J]
    atom_w = 16
    atoms = []
    for a in range(J // atom_w):
        sl = bsk[:, :, a * atom_w:(a + 1) * atom_w]
        v = np.broadcast_to(valid[None, :, a * atom_w:(a + 1) * atom_w], sl.shape)
        vals = sl[v]
        atoms.append((int(vals.min()), int(vals.max())) if vals.size else None)

    def skew_cost(w):
        return (8 * w * 1.0417 * 0.25 + 110) + (8 * w * 1.0417 * 0.5 + 110)

    skew_segs = _dp_segments(atoms, atom_w, skew_cost)

    # ---- far field plan (per row-tile) ----
    def far_cost(w):
        return (w * 1.0417 * 0.25 + 110) + (w * 1.0417 * 0.5 + 110)

    far_plans = {}
    for r in range(NT):
        n0f = P * r + 2 * P
        if n0f >= N:
            far_plans[r] = []
            continue
        na = (N - n0f) // P
        atoms_f = []
        for a in range(na):
            lo, hi = n0f + P * a, n0f + P * (a + 1)
            dmin = int((tsq[:, lo] - ts[:, P * r + P - 1]).min())
            dmax = int((tsq[:, hi - 1] - ts[:, P * r]).max())
            atoms_f.append((int(_bucket(np.array([max(dmin, 0)]))[0]),
                            int(_bucket(np.array([dmax]))[0])))
        segs = _dp_segments(atoms_f, P, far_cost)
        far_plans[r] = [(n0f + a0, n0f + a1, kmin, kmax)
                        for (a0, a1, kmin, kmax) in segs]
    return skew_segs, far_plans


def _build(ts_w_np, skew_segs, far_plans, needs_gamma_a, needs_padout):
    nc = bacc.Bacc()
    far_w = [max(0, N - (P * r + 2 * P)) for r in range(NT)]
    far_off = np.concatenate([[0], np.cumsum(far_w)]).astype(int)
    FW = int(far_off[-1])  # 2688

    d = {}
    for name, shape, dt in [
        ("xT_p", [P, 4 * N], BF16),
        ("xr_p", [P, NT * D], F32),
        ("uvqk_p", [P, 4 * E], BF16),
        ("wo_p", [P, 4 * D], BF16),
        ("bUv_row", [1, D], BF16),
        ("tsqs", [P, NT * J], F32),
        ("ntsk", [P, NT], F32),
        ("tsq_rep", [P, N], F32),
        ("near_init", [P, NT * J], F16),
        ("far_init", [P, FW], F16),
        ("smalls", [P, 48], F32),
    ]:
        d[name] = nc.dram_tensor(name, shape, dt, kind="ExternalInput")
    skew_sc = nc.dram_tensor("skew_sc", [P * NT * J], F16, kind="Internal")
    out_t = nc.dram_tensor("out", [N, D], F32, kind="ExternalOutput")

    tsw = ts_w_np.astype(np.float64)
    cks = [float(tsw[k] - tsw[k - 1]) for k in range(1, 129)]

    from contextlib import ExitStack
    with tile.TileContext(nc) as tc, ExitStack() as ctx:
        io = ctx.enter_context(tc.tile_pool(name="io", bufs=1))
        pools = ctx.enter_context(tc.tile_pool(name="work", bufs=2))
        rpool = ctx.enter_context(tc.tile_pool(name="resid", bufs=8))
        kpool = ctx.enter_context(tc.tile_pool(name="kpool", bufs=2))
        psqk = ctx.enter_context(tc.tile_pool(name="psqk", bufs=3, space="PSUM"))
        pstat = ctx.enter_context(tc.tile_pool(name="pstat", bufs=1, space="PSUM"))

        AT = mybir.AluOpType
        AF = mybir.ActivationFunctionType

        # ---- input DMAs, ordered by first use ----
        xT = io.tile([P, 4 * N], BF16, tag="xT")
        nc.sync.dma_start(xT[:, 0:2 * N], d["xT_p"][:, 0:2 * N])
        nc.sync.dma_start(xT[:, 2 * N:4 * N], d["xT_p"][:, 2 * N:4 * N])
        tsqs = io.tile([P, NT * J], F32, tag="tsqs")
        nc.sync.dma_start(tsqs[:], d["tsqs"][:])
        ntsk = io.tile([P, NT], F32, tag="ntsk")
        nc.sync.dma_start(ntsk[:], d["ntsk"][:])
        tsq_rep = io.tile([P, N], F32, tag="tsqr")
        nc.sync.dma_start(tsq_rep[:], d["tsq_rep"][:])
        near = io.tile([P, NT * J], F16, tag="near")
        nc.sync.dma_start(near[:], d["near_init"][:])
        facc = io.tile([P, FW], F16, tag="facc")
        nc.sync.dma_start(facc[:], d["far_init"][:])
        smalls = io.tile([P, 48], F32, tag="smalls")
        nc.sync.dma_start(smalls[:], d["smalls"][:])
        bUv = io.tile([1, D], BF16, tag="bUv")
        nc.sync.dma_start(bUv[:], d["bUv_row"][:])
        uvqk = io.tile([P, 4 * E], BF16, tag="uvqk")
        nc.sync.dma_start(uvqk[:], d["uvqk_p"][:])
        wo = io.tile([P, 4 * D], BF16, tag="wo")
        # smalls layout: bU_col [0:16], vscale [16:24], padout [24:32],
        # ga [32:36], bb [36:40]

        ident = io.tile([P, P], F16, tag="ident")
        make_identity(nc, ident[:])
        ots = []
        for t in range(NT):
            ot = rpool.tile([P, D], F32, tag="wot", name=f"ot{t}")
            ots.append(ot)
        ones_col = io.tile([P, 1], BF16, tag="ones_col")
        nc.vector.memset(ones_col[:], 1.0)
        ones_row16 = io.tile([1, P], F16, tag="ones_row16")
        nc.vector.memset(ones_row16[:], 1.0)
        ones_rowb = io.tile([1, P], BF16, tag="ones_rowb")
        nc.vector.memset(ones_rowb[:], 1.0)

        # ---- rel-bias prep on Act: d2 = (tsqs - ts)^2, y = ln(d2) ----
        d2 = io.tile([P, FW], F32, tag="d2")
        for g in range(NT):
            nc.scalar.activation(d2[:, J * g:J * g + J], tsqs[:, J * g:J * g + J],
                                 AF.Square, bias=ntsk[:, g:g + 1], scale=1.0)
        yk = io.tile([P, NT * J], F16, tag="yk")
        nc.scalar.activation(yk[:], d2[:, 0:NT * J], AF.Ln)
        yf = io.tile([P, FW], F16, tag="yf")

        # ---- layernorm stats of x ----
        stats = pstat.tile([P, 2 * D], F32, tag="stats", name="stats")
        for s in range(4):
            sq = pools.tile([P, N], BF16, tag="wsq", name="sq")
            nc.vector.tensor_tensor(sq[:], xT[:, N * s:N * s + N],
                                    xT[:, N * s:N * s + N], AT.mult)
            for c in range(2):
                nc.tensor.matmul(stats[0:1, 512 * c:512 * c + 512], ones_col[:],
                                 xT[:, N * s + 512 * c:N * s + 512 * c + 512],
                                 start=(s == 0), stop=(s == 3))
                nc.tensor.matmul(stats[32:33, 512 * c:512 * c + 512], ones_col[:],
                                 sq[:, 512 * c:512 * c + 512],
                                 start=(s == 0), stop=(s == 3))
        stg = io.tile([1, 2 * N], F16, tag="stg")
        tmp1 = pools.tile([1, N], F32, tag="wrow", name="tmp1")
        mu2 = pools.tile([1, N], F32, tag="wrow", name="mu2")
        murs = io.tile([P, 2 * N], F16, tag="murs")
        # per-512-chunk pipelined LN chain so xnT/proj of chunk 0 can start
        # while chunk 1's statistics are still being reduced
        for c in range(2):
            sl = slice(512 * c, 512 * c + 512)
            nc.vector.tensor_scalar_mul(stg[:, sl], stats[0:1, sl], 1.0 / D)
            nc.vector.tensor_scalar(tmp1[:, sl], stats[32:33, sl],
                                    1.0 / D, EPS, AT.mult, AT.add)
            nc.vector.tensor_tensor(mu2[:, sl], stg[:, sl], stg[:, sl], AT.mult)
            nc.vector.tensor_tensor(tmp1[:, sl], tmp1[:, sl], mu2[:, sl],
                                    AT.subtract)
            nc.scalar.activation(tmp1[:, sl], tmp1[:, sl], AF.Sqrt)
            with nc.allow_low_precision(reason="fp16 rstd is fine"):
                nc.vector.reciprocal(stg[:, N + 512 * c:N + 512 * c + 512],
                                     tmp1[:, sl])
            for cc in (512 * c, N + 512 * c):  # replicate mu-c then rs-c
                pt = psqk.tile([P, 2 * 512], F32, tag="qk", name="rep")
                nc.tensor.matmul(pt[:, 0:512], ones_row16[:],
                                 stg[:, cc:cc + 512],
                                 start=True, stop=True)
                nc.scalar.copy(out=murs[:, cc:cc + 512], in_=pt[:, 0:512])
            # xnT chunk = (xT - mu) * rs  (in place, bf16)
            for s in range(4):
                nc.vector.tensor_tensor(xT[:, N * s + 512 * c:N * s + 512 * c + 512],
                                        xT[:, N * s + 512 * c:N * s + 512 * c + 512],
                                        murs[:, sl], AT.subtract)
                nc.vector.tensor_tensor(xT[:, N * s + 512 * c:N * s + 512 * c + 512],
                                        xT[:, N * s + 512 * c:N * s + 512 * c + 512],
                                        murs[:, N + 512 * c:N + 512 * c + 512],
                                        AT.mult)

        # far-field d2/Ln (emitted after the x-LN Act work so the proj-
        # blocking chain drains first; Pool far passes start ~2us later)
        for r in range(NT):
            if far_w[r] == 0:
                continue
            n0f = P * r + 2 * P
            nc.scalar.activation(d2[:, far_off[r]:far_off[r + 1]],
                                 tsq_rep[:, n0f:N], AF.Square,
                                 bias=ntsk[:, r:r + 1], scale=1.0)
        nc.scalar.activation(yf[:], d2[:], AF.Ln)

        # ---- skew threshold passes (DVE) ----
        def near_ap(tile_, j0, w, ng):
            a = tile_[:]
            return AP(a.tensor, a.offset + j0, [[NT * J, P], [J, ng], [1, w]])

        for (j0, j1, kmin, kmax) in skew_segs:
            w = j1 - j0
            ng = 7 if j0 >= 128 else 8
            for k in range(kmin + 1, kmax + 1):
                t = kpool.tile([P, 512], F16, tag="kt")
                ta = t[:]
                tap = AP(ta.tensor, ta.offset, [[512, P], [w, ng], [1, w]])
                nc.vector.tensor_scalar(tap, near_ap(yk, j0, w, ng),
                                        float(TH * k), cks[k - 1],
                                        AT.is_ge, AT.mult)
                nc.vector.tensor_tensor(near_ap(near, j0, w, ng),
                                        near_ap(near, j0, w, ng),
                                        tap, AT.add)
        # far threshold passes (r<2 on DVE; r>=2 offloaded to idle GPSIMD,
        # with a separate scratch ring so the two chains run concurrently)
        for r in range(NT):
            eng = nc.vector if r < 2 else nc.gpsimd
            tg = "kt" if r < 2 else "ktp"
            for (a0, a1, kmin, kmax) in far_plans[r]:
                o0 = int(far_off[r] + (a0 - (P * r + 2 * P)))
                w = a1 - a0
                for k in range(kmin + 1, kmax + 1):
                    t = kpool.tile([P, 512], F16, tag=tg, name="tthr")
                    eng.tensor_scalar(t[:, :w], yf[:, o0:o0 + w],
                                      float(TH * k), cks[k - 1],
                                      AT.is_ge, AT.mult)
                    eng.tensor_tensor(facc[:, o0:o0 + w], facc[:, o0:o0 + w],
                                      t[:, :w], AT.add)

        # corner fill: groups 0..6 keep p+j<=255; group 7 keep p+j<=127
        a06 = near[:]
        nc.gpsimd.affine_select(
            out=AP(a06.tensor, a06.offset, [[NT * J, P], [J, 7], [1, J]]),
            in_=AP(a06.tensor, a06.offset, [[NT * J, P], [J, 7], [1, J]]),
            pattern=[[0, 7], [-1, J]], compare_op=AT.is_ge, fill=FILL,
            base=255, channel_multiplier=-1)
        nc.gpsimd.affine_select(
            out=near[:, 7 * J:8 * J], in_=near[:, 7 * J:8 * J],
            pattern=[[-1, J]], compare_op=AT.is_ge, fill=FILL,
            base=127, channel_multiplier=-1)

        # prefetch residual tiles on the idle Pool DMA queue
        for t in range(NT):
            nc.gpsimd.dma_start(ots[t][:], d["xr_p"][:, D * t:D * t + D])

        # unskew via DRAM round-trip
        sc = skew_sc[:].rearrange("(a b) -> a b", a=P)
        nc.sync.dma_start(sc, near[:])
        near_mn = io.tile([P, NT * J], F16, tag="near_mn")
        scf = skew_sc[:]
        nc.sync.dma_start(
            AP(near_mn[:].tensor, near_mn[:].offset, [[NT * J, P], [J, NT], [1, J]]),
            AP(scf.tensor, scf.offset, [[NT * J - 1, P], [J, NT], [1, J]]))

        # ---- projT for q,k,u tiles (order: qk first); v row-major ----
        projT = {}
        for t in [8, 12, 9, 13, 10, 14, 11, 15, 0, 1, 2, 3]:
            projT[t] = io.tile([P, N], F16, tag=f"pT{t}", name=f"pT{t}")
            for c in range(2):
                pt = psqk.tile([P, 2 * 512], F32, tag="qk", name="ptp")
                for s in range(4):
                    nc.tensor.matmul(pt[:, 0:512],
                                     uvqk[:, E * s + P * t:E * s + P * t + P],
                                     xT[:, N * s + 512 * c:N * s + 512 * c + 512],
                                     start=(s == 0), stop=(s == 3))
                nc.scalar.activation(projT[t][:, 512 * c:512 * c + 512],
                                     pt[:, 0:512],
                                     AF.Silu, bias=smalls[:, t:t + 1], scale=1.0)
        vt = [io.tile([P, D], F16, tag=f"v{r}", name=f"v{r}") for r in range(NT)]
        for r in range(NT):
            pt = psqk.tile([P, 2 * 512], F32, tag="qk", name="ptv")
            for s in range(4):
                nc.tensor.matmul(pt[:, 0:512],
                                 xT[:, N * s + P * r:N * s + P * r + P],
                                 uvqk[:, E * s + 512:E * s + 1024],
                                 start=(s == 0), stop=False)
            nc.tensor.matmul(pt[:, 0:512], ones_rowb[:], bUv[:],
                             start=False, stop=True)
            sv = pools.tile([P, D], F16, tag="wsv", name="sv")
            nc.scalar.activation(sv[:], pt[:, 0:512], AF.Silu)
            nc.vector.tensor_scalar(vt[r][:], sv[:], smalls[:, 16 + r:17 + r],
                                    None, AT.mult)

        # ---- attention per head ----
        qksil2 = [[io.tile([P, N], F16, tag=f"qs{i}_{r}", name=f"qs{i}_{r}")
                   for r in range(NT)] for i in range(2)]
        attnT = [io.tile([P, N], BF16, tag=f"aT{t}", name=f"aT{t}") for t in range(4)]

        def attnv(h, c):
            pq = 64 * (h % 2)
            qs = qksil2[h % 2]
            pa = stats2[64:128, 512 * (h % 2):512 * (h % 2) + 512]
            first = True
            for c1 in range(4 * c, 4 * c + 4):
                col = P * (c1 - 4 * c)
                for r in range(c1 + 1):
                    last = (c1 == 4 * c + 3) and (r == c1)
                    nc.tensor.matmul(pa[:, col:col + P],
                                     vt[r][:, 64 * h:64 * h + 64],
                                     qs[r][:, P * c1:P * c1 + P],
                                     start=first, stop=last,
                                     skip_group_check=True)
                    first = False
            at = attnT[h // 2]
            if h >= 6:
                nc.scalar.copy(out=at[pq:pq + 64, 512 * c:512 * c + 512],
                               in_=pa[:, 0:512])
            else:
                nc.vector.tensor_copy(out=at[pq:pq + 64, 512 * c:512 * c + 512],
                                      in_=pa[:, 0:512])

        stats2 = pstat.tile([P, 2 * D], F32, tag="stats", name="stats2")

        def qk_tile(h, r):
            qt = projT[8 + h // 2]
            kt = projT[12 + h // 2]
            pq = 64 * (h % 2)
            m0 = P * r
            w = N - m0
            wn = min(2 * P, N - m0)  # near width (256, or 128 for r=7)
            pt = psqk.tile([P, 2 * 512], F32, tag="qk", name="pt")
            # bias preload. start=True zeroes the addressed partitions'
            # whole PSUM bank, so each 512-col bank gets start=True on its
            # FIRST piece only; later pieces accumulate onto the zeroed
            # remainder.  Pieces are also split at bank boundaries.
            started = set()

            def _preload(c0, c1_, src_ap):
                bank = c0 // 512
                st = bank not in started
                started.add(bank)
                nc.tensor.matmul(pt[:, c0:c1_], ident[:], src_ap,
                                 start=st, stop=False, skip_group_check=True)

            _preload(0, wn, near_mn[:, J * r:J * r + wn])
            for (a0, a1, kmin, kmax) in far_plans[r]:
                s0 = a0
                while s0 < a1:
                    nxt_bank = m0 + (((s0 - m0) // 512) + 1) * 512
                    s1 = min(s0 + 512, a1, nxt_bank)
                    o0 = int(far_off[r] + (s0 - (m0 + 2 * P)))
                    _preload(s0 - m0, s1 - m0, facc[:, o0:o0 + (s1 - s0)])
                    s0 = s1
            # qk matmuls in <=512 column pieces
            q0 = m0
            while q0 < N:
                q1 = min(q0 + 512, N)
                nc.tensor.matmul(pt[:, q0 - m0:q1 - m0],
                                 kt[pq:pq + 64, m0:m0 + P],
                                 qt[pq:pq + 64, q0:q1],
                                 start=False, stop=True, skip_group_check=True)
                q0 = q1
            nc.scalar.activation(qksil2[h % 2][r][:, m0:N], pt[:, 0:w], AF.Silu)

        for hp in range(H // 2):
            ha, hb = 2 * hp, 2 * hp + 1
            for r in range(NT):
                qk_tile(ha, r)
                qk_tile(hb, r)
                if r == 3:
                    attnv(ha, 0)
                    attnv(hb, 0)
            attnv(ha, 1)
            attnv(hb, 1)
            # this attnT s-tile is complete; fold its LN-a stats in now
            s = hp
            for c in range(2):
                nc.tensor.matmul(stats2[0:1, 512 * c:512 * c + 512],
                                 ones_col[:],
                                 attnT[s][:, 512 * c:512 * c + 512],
                                 start=(s == 0), stop=(s == 3),
                                 skip_group_check=True)
                sqa = pools.tile([P, 512], BF16, tag="wsqa", name="sqa")
                nc.vector.tensor_tensor(sqa[:],
                                        attnT[s][:, 512 * c:512 * c + 512],
                                        attnT[s][:, 512 * c:512 * c + 512],
                                        AT.mult)
                nc.tensor.matmul(stats2[32:33, 512 * c:512 * c + 512],
                                 ones_col[:], sqa[:],
                                 start=(s == 0), stop=(s == 3),
                                 skip_group_check=True)

        # ---- layernorm of attn (over E=512, partition dim) ----
        nc.sync.dma_start(wo[:], d["wo_p"][:])
        stg2 = io.tile([1, 2 * N], F16, tag="stg2")
        tmpa = pools.tile([1, N], F32, tag="wrow", name="tmpa")
        mua2 = pools.tile([1, N], F32, tag="wrow", name="mua2")
        for c in range(2):
            sl = slice(512 * c, 512 * c + 512)
            with nc.allow_low_precision(reason="fp16 mean staging"):
                nc.scalar.activation(stg2[:, sl], stats2[0:1, sl], AF.Copy,
                                     scale=1.0 / D)
            nc.vector.tensor_scalar(tmpa[:, sl], stats2[32:33, sl],
                                    1.0 / D, EPS, AT.mult, AT.add)
            nc.scalar.activation(mua2[:, sl], stg2[:, sl], AF.Square)
            nc.vector.tensor_tensor(tmpa[:, sl], tmpa[:, sl], mua2[:, sl],
                                    AT.subtract)
            nc.scalar.activation(tmpa[:, sl], tmpa[:, sl], AF.Sqrt)
            with nc.allow_low_precision(reason="fp16 rstd is fine"):
                nc.vector.reciprocal(stg2[:, N + 512 * c:N + 512 * c + 512],
                                     tmpa[:, sl])
        muars = io.tile([P, 2 * N], F16, tag="muars")
        for c in range(4):
            pt = psqk.tile([P, 2 * 512], F32, tag="qk", name="rep")
            nc.tensor.matmul(pt[:, 0:512], ones_row16[:],
                             stg2[:, 512 * c:512 * c + 512],
                             start=True, stop=True)
            nc.scalar.copy(out=muars[:, 512 * c:512 * c + 512], in_=pt[:, 0:512])
        # prod = u * LN_a(attn) in attnT layout (bf16), chunked so the
        # first outproj tiles can start before the whole tail finishes
        for c in range(2):
            for s in range(4):
                sl = slice(512 * c, 512 * c + 512)
                nc.vector.tensor_tensor(attnT[s][:, sl], attnT[s][:, sl],
                                        muars[:, 512 * c:512 * c + 512],
                                        AT.subtract)
                nc.vector.tensor_tensor(attnT[s][:, sl], attnT[s][:, sl],
                                        muars[:, N + 512 * c:N + 512 * c + 512],
                                        AT.mult)
                if needs_gamma_a:
                    nc.vector.tensor_scalar(attnT[s][:, sl], attnT[s][:, sl],
                                            smalls[:, 32 + s:33 + s],
                                            smalls[:, 36 + s:37 + s],
                                            AT.mult, AT.add)
                nc.vector.tensor_tensor(attnT[s][:, sl], attnT[s][:, sl],
                                        projT[s][:, sl], AT.mult)

        # ---- output projection + residual (xr already includes b_o) ----
        for t in range(NT):
            po = psqk.tile([P, 2 * 512], F32, tag="qk", name="outp")
            for s in range(4):
                nc.tensor.matmul(po[:, 0:512], attnT[s][:, P * t:P * t + P],
                                 wo[:, D * s:D * s + D],
                                 start=(s == 0), stop=(s == 3))
            ot = ots[t]
            nc.vector.tensor_tensor(ot[:], po[:, 0:512], ot[:], AT.add)
            if needs_padout:
                nc.vector.tensor_scalar(ot[:], ot[:], smalls[:, 24 + t:25 + t],
                                        None, AT.mult)
            nc.sync.dma_start(out_t[P * t:P * t + P, :], ot[:])

    nc.compile()
    return nc


def _prep_inputs(inputs):
    x = np.asarray(inputs["x"], dtype=np.float32)
    ts = np.asarray(inputs["timestamps"]).astype(np.int64)
    pad = np.asarray(inputs["pad_mask"]).astype(np.float32)
    uvqk = np.asarray(inputs["uvqk"], dtype=np.float32)
    W_o = np.asarray(inputs["W_o"], dtype=np.float32)
    b_o = np.asarray(inputs["b_o"], dtype=np.float32)
    gx = np.asarray(inputs["gamma_x"], dtype=np.float32)
    bx = np.asarray(inputs["beta_x"], dtype=np.float32)
    ga = np.asarray(inputs["gamma_a"], dtype=np.float32)
    ba = np.asarray(inputs["beta_a"], dtype=np.float32)
    ts_w = np.asarray(inputs["ts_w"], dtype=np.float32)
    pos_w = np.asarray(inputs["pos_w"], dtype=np.float32)

    tsq = np.concatenate([ts[:, 1:], ts[:, -1:]], axis=1)  # [B, N]
    skew_segs, far_plans = _plan(ts, tsq)
    needs_gamma_a = bool(np.any(ga != 1.0) or np.any(ba != 0.0))
    needs_padout = bool(np.any(pad != 0.0))

    uvqk_g = (uvqk * gx[:, None]).astype(NPBF16)  # [D, E]
    uvqk_p = np.ascontiguousarray(
        uvqk_g.reshape(4, P, E).transpose(1, 0, 2).reshape(P, 4 * E))
    bU = bx @ uvqk  # [E]
    bU_col = np.ascontiguousarray(bU.reshape(E // P, P).T)  # [P, 16]
    wo_p = np.ascontiguousarray(
        W_o.astype(NPBF16).reshape(4, P, D).transpose(1, 0, 2).reshape(P, 4 * D))

    far_w = [max(0, N - (P * r + 2 * P)) for r in range(NT)]
    far_off = np.concatenate([[0], np.cumsum(far_w)]).astype(int)
    FW = int(far_off[-1])

    # near_init: col g*J + j -> pos_w[j + N-1] + ts_w[kmin(skew seg of j)]
    ninit_row = np.zeros(NT * J, np.float32)
    for g in range(NT):
        ninit_row[g * J:(g + 1) * J] = pos_w[(N - 1) - np.arange(J)]
    for (j0, j1, kmin, kmax) in skew_segs:
        for g in range(NT):
            ninit_row[g * J + j0:g * J + j1] += ts_w[kmin]
    near_init = np.broadcast_to(ninit_row.astype(NPF16), (P, NT * J)).copy()

    # far_init: per row-tile [m, n] layout
    far_init = np.zeros((P, FW), NPF16)
    for r in range(NT):
        if far_w[r] == 0:
            continue
        m = P * r + np.arange(P)[:, None]
        n0f = P * r + 2 * P
        nn = np.arange(n0f, N)[None, :]
        block = pos_w[m - nn + (N - 1)].astype(np.float32)
        for (a0, a1, kmin, kmax) in far_plans[r]:
            block[:, a0 - n0f:a1 - n0f] += ts_w[kmin]
        far_init[:, far_off[r]:far_off[r + 1]] = block.astype(NPF16)

    # tsqs rolling window index (per batch)
    m = np.arange(N)
    idx = np.minimum(m[:, None] + np.arange(J)[None, :], N - 1)  # [N, J]

    smalls_shared = np.zeros((P, 48), np.float32)
    smalls_shared[:, 0:16] = bU_col
    smalls_shared[:, 32:36] = ga.reshape(4, P).T
    smalls_shared[:, 36:40] = ba.reshape(4, P).T

    per_core = []
    for b in range(B):
        tsqs_b = tsq[b][idx].astype(np.float32)  # [N, J]
        tsqs_p = np.ascontiguousarray(
            tsqs_b.reshape(NT, P, J).transpose(1, 0, 2).reshape(P, NT * J))
        sm = smalls_shared.copy()
        sm[:, 16:24] = ((1.0 - pad[b]) / N).reshape(NT, P).T
        sm[:, 24:32] = (1.0 - pad[b]).reshape(NT, P).T
        per_core.append({
            "xT_p": np.ascontiguousarray(
                x[b].T.astype(NPBF16).reshape(4, P, N).transpose(1, 0, 2)
                .reshape(P, 4 * N)),
            "xr_p": np.ascontiguousarray(
                (x[b] + b_o[None, :]).reshape(NT, P, D).transpose(1, 0, 2)
                .reshape(P, NT * D)),
            "uvqk_p": uvqk_p,
            "wo_p": wo_p,
            "bUv_row": np.ascontiguousarray(bU[512:1024].reshape(1, D)).astype(NPBF16),
            "tsqs": tsqs_p,
            "ntsk": np.ascontiguousarray(-ts[b].astype(np.float32).reshape(NT, P).T),
            "tsq_rep": np.broadcast_to(tsq[b].astype(np.float32), (P, N)).copy(),
            "near_init": near_init,
            "far_init": far_init,
            "smalls": sm,
        })
    return per_core, (skew_segs, far_plans, needs_gamma_a, needs_padout, ts_w)


def kernel(**inputs):
    from concourse.bass_utils import run_bass_kernel_spmd

    per_core, (skew_segs, far_plans, nga, npo, ts_w) = _prep_inputs(inputs)
    key = (tuple(skew_segs), tuple((r, tuple(v)) for r, v in far_plans.items()),
           nga, npo, ts_w.tobytes())
    if key not in _cache:
        _cache.clear()
        _cache[key] = _build(ts_w, skew_segs, far_plans, nga, npo)
    nc = _cache[key]
    res = run_bass_kernel_spmd(nc, per_core, list(range(B)))
    out = np.stack([res.results[b]["out"] for b in range(B)], axis=0)
    return out.astype(np.float32)
